# revision 27
# baseline (speedup 1.0000x reference)
"""Sparse (diagonal-masked) multi-head attention on 8 Trainium2 NeuronCores.

Reference computation (B=2, S=2048, D=1024, H=16, HD=64):
    qh = q @ Wq.T + bq   (similarly kh, vh), per-head scores = qh @ kh.T / 8,
    s = softmax(scores); s = s * (1 - I); s = s / (s.sum(-1) + 1e-6)
    out = (s @ vh) @ Wo.T + bo;   returns (out, s)

Sharding: core c handles batch b = c//4 and head-group g = c%4 (4 heads, 256
channels).  Q/K/V/O projection weights are column/row-sharded over heads.  The
[B,H,S,S] probability tensor partitions over the head axis; `out` is
sum-sharded over the 4 cores of a batch (host reduces + adds bias).

Per-core kernel math (unstabilized softmax is safe: |scores/8| < ~6):
    p = exp(scores/8);  Zm[q] = sum_l p[q,l] (l != q)
    D[q] = (1+eps) * Zm[q]          (exactly Z - p_diag + eps*Z of the ref)
    s[q,l] = p[q,l] * (l != q) / D[q]
The transposed probabilities pT (needed as the moving operand of the P.V
matmul) come from a second scores matmul in [l,q] layout; exp() is fused into
the mandatory PSUM->SBUF copy on the scalar engine in both layouts.  Zm comes
for free from a ones-column appended to V in the P.V matmul.  1/D enters the
[q,l[ side as a per-partition bias (-ln D) of the fused exp.
"""

import os
import sys

import numpy as np

sys.path.insert(0, "/opt/trn_rl_repo")

import concourse.bass as bass  # noqa: E402
from concourse import bacc  # noqa: E402
import concourse.mybir as mybir  # noqa: E402
import concourse.tile as tile  # noqa: E402

F32 = mybir.dt.float32
F32R = mybir.dt.float32r
BF16 = mybir.dt.bfloat16
EPS = 1e-6
# matmul-operand dtype: f32 (2 cyc/row), f32r (1.5), bf16 (1, FWL weight loads)
KERNEL_DT = os.environ.get("KERNEL_DT", os.environ.get("KERNEL_F32R", "1") == "1" and "f32r" or "f32")

B, S, D, H = 2, 2048, 1024, 16
HD = D // H          # 64
HPC = 4              # heads per core
CPC = HPC * HD       # channels per core = 256
AUG = HPC * (HD + 1)  # v channels + one ones-column per head = 260


def build_core_module(s_len=S, d_model=D, mdt=None):
    """Bass program for one core: 4 heads of one batch. SPMD across 8 cores."""
    nc = bacc.Bacc()
    mdt = mdt or KERNEL_DT
    MT = {"f32": F32, "f32r": F32R, "bf16": BF16}[mdt]
    KC = d_model // 128   # contraction chunks for projections (8)
    NT = s_len // 128     # 128-row tiles along sequence (16)
    W = min(512, s_len)   # psum chunk width for scores (1 bank)
    NW = s_len // W       # chunks per full row (2)
    NQ = s_len // 512     # 512-wide matmul chunks (4)
    XG = max(1, s_len // 1024)  # x-chunk column groups in projections

    qT = nc.declare_dram_parameter("qT", [d_model, s_len], MT, isOutput=False)
    kT = nc.declare_dram_parameter("kT", [d_model, s_len], MT, isOutput=False)
    vT = nc.declare_dram_parameter("vT", [d_model, s_len], MT, isOutput=False)
    wqT = nc.declare_dram_parameter("wqT", [d_model, CPC], MT, isOutput=False)
    wkT = nc.declare_dram_parameter("wkT", [d_model, CPC], MT, isOutput=False)
    wvT = nc.declare_dram_parameter("wvT", [d_model, AUG], MT, isOutput=False)
    woT = nc.declare_dram_parameter("woT", [CPC, d_model], MT, isOutput=False)
    bqv = nc.declare_dram_parameter("bqv", [CPC], F32, isOutput=False)
    bkv = nc.declare_dram_parameter("bkv", [CPC], F32, isOutput=False)
    bvb = nc.declare_dram_parameter("bvb", [AUG], F32, isOutput=False)

    s_part = nc.declare_dram_parameter("s_part", [HPC, s_len, s_len], F32, isOutput=True)
    out_p = nc.declare_dram_parameter("out_p", [s_len, d_model], F32, isOutput=True)

    AF = mybir.ActivationFunctionType

    lowp = nc.allow_low_precision("fp32r matmul operands")
    lowp.__enter__()
    with tile.TileContext(nc) as tc:
        import contextlib

        with contextlib.ExitStack() as ctx:
            main = ctx.enter_context(tc.tile_pool(name="main", bufs=1))

            # ---- constants -------------------------------------------------
            inv_mask = main.tile([128, 128], F32)  # 1 - I
            nc.gpsimd.memset(inv_mask, 1.0)
            nc.gpsimd.affine_select(
                out=inv_mask, in_=inv_mask,
                compare_op=mybir.AluOpType.not_equal, fill=0.0,
                base=0, pattern=[[-1, 128]], channel_multiplier=1,
            )
            ones_row = main.tile([1, 128], F32)  # lhsT for broadcasts / rhs ones
            nc.vector.memset(ones_row, 1.0)
            ident = main.tile([128, 128], F32)  # I, rhs of transpose matmuls
            nc.gpsimd.memset(ident, 0.0)
            nc.gpsimd.affine_select(
                out=ident, in_=ident,
                compare_op=mybir.AluOpType.not_equal, fill=1.0,
                base=0, pattern=[[-1, 128]], channel_multiplier=1,
            )

            # ---- persistent activations -----------------------------------
            # qhT/khT: [e, s] per-head-transposed projections; channel
            # 128*m + p lives at (partition p, free index m).
            qhT = main.tile([128, 2, s_len], MT)
            khT = main.tile([128, 2, s_len], MT)
            # vh: natural [s, e'] layout + per-head ones column.
            # Head h occupies cols 65h..65h+63; col 65h+64: ones (even h) /
            # col 65h: ones, 65h+1.. v (odd h).
            vh = main.tile([128, NT, AUG], MT)
            # attnT pair-packed: pair m holds head 2m (rows 0:64) and head
            # 2m+1 (rows 64:128), [e', q] layout.
            attnT = main.tile([128, 2, s_len], MT)
            wo_sb = main.tile([128, 2, d_model], MT)
            nc.sync.dma_start(
                out=wo_sb, in_=woT[:].rearrange("(m p) d -> p m d", p=128)
            )
            bq_sb = main.tile([128, 2], F32)
            nc.gpsimd.dma_start(out=bq_sb, in_=bqv[:].rearrange("(m p) -> p m", p=128))
            bk_sb = main.tile([128, 2], F32)
            nc.gpsimd.dma_start(out=bk_sb, in_=bkv[:].rearrange("(m p) -> p m", p=128))
            bv_sb = main.tile([128, AUG], F32)
            bvb_ap = bvb[:]
            nc.gpsimd.dma_start(
                out=bv_sb,
                in_=bass.AP(tensor=bvb_ap.tensor, offset=bvb_ap.offset,
                            ap=[[0, 128]] + list(bvb_ap.ap)),
            )

            # ---- projections ----------------------------------------------
            with tc.tile_pool(name="wpool", bufs=1) as wp, \
                 tc.tile_pool(name="xpool", bufs=3) as xp, \
                 tc.tile_pool(name="ppj", bufs=8, space="PSUM") as pp:
                wq_sb = wp.tile([128, KC, CPC], MT)
                nc.sync.dma_start(
                    out=wq_sb, in_=wqT[:].rearrange("(c p) e -> p c e", p=128))
                wk_sb = wp.tile([128, KC, CPC], MT)
                nc.sync.dma_start(
                    out=wk_sb, in_=wkT[:].rearrange("(c p) e -> p c e", p=128))
                wv_sb = wp.tile([128, KC, AUG], MT)
                nc.sync.dma_start(
                    out=wv_sb, in_=wvT[:].rearrange("(c p) e -> p c e", p=128))

                GW = s_len // XG  # columns per x-chunk group
                # q/k projections: out[e, s] = sum_d W[d, e] * xT[d, s]
                for xdram, w_sb, b_sb, outT in (
                    (qT, wq_sb, bq_sb, qhT), (kT, wk_sb, bk_sb, khT),
                ):
                    for grp in range(XG):
                        psl = []
                        for m in range(2):
                            for n in range(GW // 512):
                                pt = pp.tile([128, 512], F32, tag="pj",
                                             name=f"pj_{m}_{n}")
                                psl.append(pt)
                        for kk in range(KC):
                            xc = xp.tile([128, GW], MT, tag="xc", name="xc")
                            nc.sync.dma_start(
                                out=xc,
                                in_=xdram[kk * 128:(kk + 1) * 128,
                                          grp * GW:(grp + 1) * GW])
                            for m in range(2):
                                for n in range(GW // 512):
                                    nc.tensor.matmul(
                                        psl[m * (GW // 512) + n],
                                        lhsT=w_sb[:, kk, m * 128:(m + 1) * 128],
                                        rhs=xc[:, n * 512:(n + 1) * 512],
                                        start=(kk == 0), stop=(kk == KC - 1))
                        for m in range(2):
                            for n in range(GW // 512):
                                # DVE drain (keeps ACT free for the exps)
                                nc.vector.tensor_scalar_add(
                                    outT[:, m, grp * GW + n * 512:
                                         grp * GW + (n + 1) * 512],
                                    psl[m * (GW // 512) + n],
                                    b_sb[:, m:m + 1])
                # v projection: out[s, e'] = sum_d vT[d, s] * Wv[d, e'] + b
                for grp in range(XG):
                    pvl = [pp.tile([128, AUG], F32, tag="pj", name=f"pv_{st}")
                           for st in range(GW // 128)]
                    for kk in range(KC):
                        xc = xp.tile([128, GW], MT, tag="xc", name="xcv")
                        nc.sync.dma_start(
                            out=xc, in_=vT[kk * 128:(kk + 1) * 128,
                                           grp * GW:(grp + 1) * GW])
                        for st in range(GW // 128):
                            nc.tensor.matmul(
                                pvl[st],
                                lhsT=xc[:, st * 128:(st + 1) * 128],
                                rhs=wv_sb[:, kk, :],
                                start=(kk == 0), stop=(kk == KC - 1))
                    for st in range(GW // 128):
                        nc.vector.tensor_add(
                            vh[:, grp * (GW // 128) + st, :], pvl[st], bv_sb)

            # ---- attention -------------------------------------------------
            with tc.tile_pool(name="apool", bufs=1) as ap, \
                 tc.tile_pool(name="patt", bufs=1, space="PSUM") as pa:
                HW2 = s_len // 2          # q-half width for the B side
                CB = min(512, HW2)        # B-side psum chunk
                NCB = HW2 // CB
                for hp in range(2):
                    h0, h1 = 2 * hp, 2 * hp + 1
                    # ---- B side, head pair, two q-half passes ----------
                    # The two heads' K=64 scores matmuls sit at PE row
                    # groups 0 and 64 (auto tile_position) and issue
                    # back-to-back -> they execute concurrently.
                    z_ts, raw_ts = [], []
                    for hh in (h0, h1):
                        z_ts.append(ap.tile([65, s_len], F32, tag="zt",
                                            bufs=2, name=f"z_t{hh % 2}"))
                        raw_ts.append(ap.tile([64, s_len], F32, tag="ah",
                                              bufs=2, name=f"raw_t{hh % 2}"))
                    for half in range(2):
                        q0 = half * HW2
                        attn0 = pa.tile([65, HW2], F32, tag="attn", bufs=2,
                                        name="attn0")
                        attn1 = pa.tile([65, HW2], F32, tag="attn", bufs=2,
                                        name="attn1")
                        for lt in range(NT):
                            pT0 = ap.tile([128, HW2], MT, tag="pT", bufs=4,
                                          name="pT0")
                            pT1 = ap.tile([128, HW2], MT, tag="pT", bufs=4,
                                          name="pT1")
                            for n in range(NCB):
                                sc0 = pa.tile([128, CB], F32, tag="sc",
                                              bufs=4, name="sc0")
                                sc1 = pa.tile([128, CB], F32, tag="sc",
                                              bufs=4, name="sc1")
                                cc = q0 + n * CB
                                nc.tensor.matmul(
                                    sc0,
                                    lhsT=khT[0:64, hp, lt * 128:(lt + 1) * 128],
                                    rhs=qhT[0:64, hp, cc:cc + CB],
                                    start=True, stop=True)
                                nc.tensor.matmul(
                                    sc1,
                                    lhsT=khT[64:128, hp, lt * 128:(lt + 1) * 128],
                                    rhs=qhT[64:128, hp, cc:cc + CB],
                                    start=True, stop=True)
                                nc.scalar.activation(
                                    pT0[:, n * CB:(n + 1) * CB], sc0,
                                    AF.Exp, scale=0.125)
                                nc.scalar.activation(
                                    pT1[:, n * CB:(n + 1) * CB], sc1,
                                    AF.Exp, scale=0.125)
                            if q0 <= lt * 128 < q0 + HW2:
                                col = lt * 128 - q0
                                nc.vector.tensor_mul(
                                    pT0[:, col:col + 128],
                                    pT0[:, col:col + 128], inv_mask)
                                nc.vector.tensor_mul(
                                    pT1[:, col:col + 128],
                                    pT1[:, col:col + 128], inv_mask)
                            for n in range(HW2 // 512 if HW2 >= 512 else 1):
                                w2 = min(512, HW2)
                                nc.tensor.matmul(
                                    attn0[0:65, n * w2:(n + 1) * w2],
                                    lhsT=vh[:, lt, h0 * 65:(h0 + 1) * 65],
                                    rhs=pT0[:, n * w2:(n + 1) * w2],
                                    start=(lt == 0), stop=(lt == NT - 1))
                                nc.tensor.matmul(
                                    attn1[0:65, n * w2:(n + 1) * w2],
                                    lhsT=vh[:, lt, h1 * 65:(h1 + 1) * 65],
                                    rhs=pT1[:, n * w2:(n + 1) * w2],
                                    start=(lt == 0), stop=(lt == NT - 1))
                        # drain this q-half out of PSUM promptly
                        for i_h, attn_ps in ((0, attn0), (1, attn1)):
                            nc.vector.tensor_scalar_mul(
                                z_ts[i_h][64:65, q0:q0 + HW2],
                                attn_ps[64:65, :], 1.0 + EPS)
                            nc.vector.tensor_copy(
                                raw_ts[i_h][:, q0:q0 + HW2], attn_ps[0:64, :])

                    # ---- stats + attnT scale, per head -----------------
                    neg_lnds = []
                    for i_h, hh in enumerate((h0, h1)):
                        odd = hh % 2
                        z_t, raw_t = z_ts[i_h], raw_ts[i_h]
                        d_row = ap.tile([1, s_len], F32, tag="d0", bufs=1,
                                        name="d_row")
                        nc.sync.dma_start(out=d_row, in_=z_t[64:65, :])
                        # D -> partition layout; [128, NT] reciprocal (a
                        # [1, S] reciprocal runs 13us on one DVE lane)
                        dp_ps = pa.tile([128, CB], F32, tag="sc", bufs=4,
                                        name="dp_ps")
                        for t in range(NT):
                            nc.tensor.matmul(
                                dp_ps[:, t:t + 1],
                                lhsT=d_row[:, t * 128:(t + 1) * 128],
                                rhs=ones_row[:, 0:1], start=True, stop=True)
                        rdp = ap.tile([128, NT], F32, tag="rdp", bufs=2,
                                      name="rdp")
                        nc.vector.reciprocal(rdp, dp_ps[:, :NT])
                        neg_lnd = ap.tile([128, NT], F32, tag="nld", bufs=2,
                                          name="neg_lnd")
                        nc.scalar.activation(neg_lnd, rdp, AF.Ln)  # ln(1/D)
                        neg_lnds.append(neg_lnd)
                        # 1/D back to free layout for the 64-row broadcast
                        rd_row = ap.tile([1, s_len], F32, tag="rd", bufs=1,
                                         name="rd_row")
                        for hf in range(s_len // CB):
                            rf_ps = pa.tile([1, CB], F32, tag="sc", bufs=4,
                                            name="rf_ps")
                            for t8 in range(CB // 128):
                                t = hf * (CB // 128) + t8
                                nc.tensor.matmul(
                                    rf_ps[0:1, t8 * 128:(t8 + 1) * 128],
                                    lhsT=rdp[:, t:t + 1], rhs=ident,
                                    start=True, stop=True)
                            nc.vector.tensor_copy(
                                rd_row[:, hf * CB:(hf + 1) * CB], rf_ps)
                        bc_sb = ap.tile([64, s_len], F32, tag="bc", bufs=2,
                                        name="bc_sb")
                        for n in range(NQ):
                            bc_ps = pa.tile([64, 512], F32, tag="sc", bufs=4,
                                            name="bc_ps")
                            nc.tensor.matmul(
                                bc_ps, lhsT=ones_row[:, :64],
                                rhs=rd_row[:, n * 512:(n + 1) * 512],
                                start=True, stop=True)
                            nc.vector.tensor_copy(
                                bc_sb[:, n * 512:(n + 1) * 512], bc_ps)
                        if not odd:
                            nc.vector.tensor_mul(
                                attnT[0:64, hp, :], raw_t, bc_sb)
                        else:
                            ah_t = ap.tile([64, s_len], MT, tag="ah2", bufs=1,
                                           name="ah_t")
                            nc.vector.tensor_mul(ah_t, raw_t, bc_sb)
                            # DVE is lane-locked; DMA crosses partitions
                            nc.sync.dma_start(
                                out=attnT[64:128, hp, :], in_=ah_t)

                    # ---- A side, head pair, packed ---------------------
                    for qt in range(NT):
                        s_t0 = ap.tile([128, s_len], F32, tag="st", bufs=4,
                                       name="s_t0")
                        s_t1 = ap.tile([128, s_len], F32, tag="st", bufs=4,
                                       name="s_t1")
                        for n in range(NQ):
                            sa0 = pa.tile([128, 512], F32, tag="sc", bufs=4,
                                          name="sa0")
                            sa1 = pa.tile([128, 512], F32, tag="sc", bufs=4,
                                          name="sa1")
                            nc.tensor.matmul(
                                sa0,
                                lhsT=qhT[0:64, hp, qt * 128:(qt + 1) * 128],
                                rhs=khT[0:64, hp, n * 512:(n + 1) * 512],
                                start=True, stop=True)
                            nc.tensor.matmul(
                                sa1,
                                lhsT=qhT[64:128, hp, qt * 128:(qt + 1) * 128],
                                rhs=khT[64:128, hp, n * 512:(n + 1) * 512],
                                start=True, stop=True)
                            nc.scalar.activation(
                                s_t0[:, n * 512:(n + 1) * 512], sa0,
                                AF.Exp, scale=0.125,
                                bias=neg_lnds[0][:, qt:qt + 1])
                            nc.scalar.activation(
                                s_t1[:, n * 512:(n + 1) * 512], sa1,
                                AF.Exp, scale=0.125,
                                bias=neg_lnds[1][:, qt:qt + 1])
                        nc.vector.tensor_mul(
                            s_t0[:, qt * 128:(qt + 1) * 128],
                            s_t0[:, qt * 128:(qt + 1) * 128], inv_mask)
                        nc.vector.tensor_mul(
                            s_t1[:, qt * 128:(qt + 1) * 128],
                            s_t1[:, qt * 128:(qt + 1) * 128], inv_mask)
                        nc.sync.dma_start(
                            out=s_part[h0, qt * 128:(qt + 1) * 128, :],
                            in_=s_t0)
                        nc.sync.dma_start(
                            out=s_part[h1, qt * 128:(qt + 1) * 128, :],
                            in_=s_t1)

                # ---- output projection --------------------------------------
                for qt in range(NT):
                    o_t = ap.tile([128, d_model], F32, tag="st", bufs=3,
                                  name="o_t")
                    for n2 in range(d_model // 512):
                        op_ps = pa.tile([128, 512], F32, tag="sc", bufs=4,
                                        name="op_ps")
                        for m in range(2):
                            nc.tensor.matmul(
                                op_ps,
                                lhsT=attnT[:, m, qt * 128:(qt + 1) * 128],
                                rhs=wo_sb[:, m, n2 * 512:(n2 + 1) * 512],
                                start=(m == 0), stop=(m == 1))
                        nc.vector.tensor_copy(
                            o_t[:, n2 * 512:(n2 + 1) * 512], op_ps)
                    nc.sync.dma_start(
                        out=out_p[qt * 128:(qt + 1) * 128, :], in_=o_t)

    lowp.__exit__(None, None, None)
    nc.finalize()
    return nc


_CACHE: dict = {}


def _get_module():
    if "nc" not in _CACHE:
        _CACHE["nc"] = build_core_module()
    return _CACHE["nc"]


def _shard_inputs(q, k, v, Wq, bq, Wk, bk, Wv, bv, Wo, bo):
    """Build the 8 per-core input maps (host-side transposes/slices)."""
    f = np.float32
    if KERNEL_DT == "bf16":
        import ml_dtypes
        mf = ml_dtypes.bfloat16
    else:
        mf = np.float32
    in_maps = []
    for c in range(8):
        b, g = divmod(c, 4)
        sel = slice(g * CPC, (g + 1) * CPC)
        wv_aug = np.zeros((D, AUG), f)
        bv_aug = np.zeros(AUG, f)
        for hh in range(HPC):
            rows = Wv[g * CPC + hh * HD:g * CPC + (hh + 1) * HD, :]  # [64, D]
            bl = bv[g * CPC + hh * HD:g * CPC + (hh + 1) * HD]
            wv_aug[:, 65 * hh:65 * hh + 64] = rows.T
            bv_aug[65 * hh:65 * hh + 64] = bl
            bv_aug[65 * hh + 64] = 1.0  # ones column -> Zm row of the PV psum
        in_maps.append({
            "qT": np.ascontiguousarray(q[b].T, mf),
            "kT": np.ascontiguousarray(k[b].T, mf),
            "vT": np.ascontiguousarray(v[b].T, mf),
            "wqT": np.ascontiguousarray(Wq[sel, :].T, mf),
            "wkT": np.ascontiguousarray(Wk[sel, :].T, mf),
            "wvT": wv_aug.astype(mf),
            "woT": np.ascontiguousarray(Wo[:, sel].T, mf),
            "bqv": np.ascontiguousarray(bq[sel], f),
            "bkv": np.ascontiguousarray(bk[sel], f),
            "bvb": bv_aug,
        })
    return in_maps


def kernel(q, k, v, Wq, bq, Wk, bk, Wv, bv, Wo, bo):
    q, k, v = (np.asarray(x, np.float32) for x in (q, k, v))
    Wq, bq, Wk, bk, Wv, bv, Wo, bo = (
        np.asarray(x, np.float32) for x in (Wq, bq, Wk, bk, Wv, bv, Wo, bo))

    from concourse.bass_utils import run_bass_kernel_spmd

    nc = _get_module()
    in_maps = _shard_inputs(q, k, v, Wq, bq, Wk, bk, Wv, bv, Wo, bo)
    res = run_bass_kernel_spmd(nc, in_maps, core_ids=list(range(8)))
    _CACHE["last_res"] = res  # exec_time_ns/profile when BASS_TRACE=1
    results = res.results

    s_full = np.empty((B, H, S, S), np.float32)
    out = np.empty((B, S, D), np.float32)
    for b in range(B):
        acc = None
        for g in range(4):
            r = results[b * 4 + g]
            s_full[b, g * HPC:(g + 1) * HPC] = r["s_part"]
            acc = r["out_p"].astype(np.float32) if acc is None else acc + r["out_p"]
        out[b] = acc + bo[None, :]
    return out, s_full


# head-group order within a core maps to global heads g*4 + hh


# revision 28
# speedup vs baseline: 1.0468x; 1.0468x over previous
"""Sparse (diagonal-masked) multi-head attention on 8 Trainium2 NeuronCores.

Reference computation (B=2, S=2048, D=1024, H=16, HD=64):
    qh = q @ Wq.T + bq   (similarly kh, vh), per-head scores = qh @ kh.T / 8,
    s = softmax(scores); s = s * (1 - I); s = s / (s.sum(-1) + 1e-6)
    out = (s @ vh) @ Wo.T + bo;   returns (out, s)

Sharding: core c handles batch b = c//4 and head-group g = c%4 (4 heads, 256
channels).  Q/K/V/O projection weights are column/row-sharded over heads.  The
[B,H,S,S] probability tensor partitions over the head axis; `out` is
sum-sharded over the 4 cores of a batch (host reduces + adds bias).

Per-core kernel math (unstabilized softmax is safe: |scores/8| < ~6):
    p = exp(scores/8);  Zm[q] = sum_l p[q,l] (l != q)
    D[q] = (1+eps) * Zm[q]          (exactly Z - p_diag + eps*Z of the ref)
    s[q,l] = p[q,l] * (l != q) / D[q]
The transposed probabilities pT (needed as the moving operand of the P.V
matmul) come from a second scores matmul in [l,q] layout; exp() is fused into
the mandatory PSUM->SBUF copy on the scalar engine in both layouts.  Zm comes
for free from a ones-column appended to V in the P.V matmul.  1/D enters the
[q,l[ side as a per-partition bias (-ln D) of the fused exp.
"""

import os
import sys

import numpy as np

sys.path.insert(0, "/opt/trn_rl_repo")

import concourse.bass as bass  # noqa: E402
from concourse import bacc  # noqa: E402
import concourse.mybir as mybir  # noqa: E402
import concourse.tile as tile  # noqa: E402

F32 = mybir.dt.float32
F32R = mybir.dt.float32r
BF16 = mybir.dt.bfloat16
EPS = 1e-6
# matmul-operand dtype: f32 (2 cyc/row), f32r (1.5), bf16 (1, FWL weight loads)
KERNEL_DT = os.environ.get("KERNEL_DT", os.environ.get("KERNEL_F32R", "1") == "1" and "f32r" or "f32")
# "hybrid": scores path f32r (s output keeps ~4e-4), P.V/out-proj bf16

B, S, D, H = 2, 2048, 1024, 16
HD = D // H          # 64
HPC = 4              # heads per core
CPC = HPC * HD       # channels per core = 256
AUG = HPC * (HD + 1)  # v channels + one ones-column per head = 260


def build_core_module(s_len=S, d_model=D, mdt=None):
    """Bass program for one core: 4 heads of one batch. SPMD across 8 cores."""
    nc = bacc.Bacc()
    mdt = mdt or KERNEL_DT
    MT = {"f32": F32, "f32r": F32R, "bf16": BF16, "hybrid": F32R}[mdt]
    VT = BF16 if mdt in ("bf16", "hybrid") else MT  # P.V / out-proj operands
    KC = d_model // 128   # contraction chunks for projections (8)
    NT = s_len // 128     # 128-row tiles along sequence (16)
    W = min(512, s_len)   # psum chunk width for scores (1 bank)
    NW = s_len // W       # chunks per full row (2)
    NQ = s_len // 512     # 512-wide matmul chunks (4)
    XG = max(1, s_len // 1024)  # x-chunk column groups in projections

    qT = nc.declare_dram_parameter("qT", [d_model, s_len], MT, isOutput=False)
    kT = nc.declare_dram_parameter("kT", [d_model, s_len], MT, isOutput=False)
    vT = nc.declare_dram_parameter("vT", [d_model, s_len], MT, isOutput=False)
    wqT = nc.declare_dram_parameter("wqT", [d_model, CPC], MT, isOutput=False)
    wkT = nc.declare_dram_parameter("wkT", [d_model, CPC], MT, isOutput=False)
    wvT = nc.declare_dram_parameter("wvT", [d_model, AUG], MT, isOutput=False)
    woT = nc.declare_dram_parameter("woT", [CPC, d_model], VT, isOutput=False)
    bqv = nc.declare_dram_parameter("bqv", [CPC], F32, isOutput=False)
    bkv = nc.declare_dram_parameter("bkv", [CPC], F32, isOutput=False)
    bvb = nc.declare_dram_parameter("bvb", [AUG], F32, isOutput=False)

    s_part = nc.declare_dram_parameter("s_part", [HPC, s_len, s_len], F32, isOutput=True)
    out_p = nc.declare_dram_parameter("out_p", [s_len, d_model], F32, isOutput=True)

    AF = mybir.ActivationFunctionType

    lowp = nc.allow_low_precision("fp32r matmul operands")
    lowp.__enter__()
    with tile.TileContext(nc) as tc:
        import contextlib

        with contextlib.ExitStack() as ctx:
            main = ctx.enter_context(tc.tile_pool(name="main", bufs=1))

            # ---- constants -------------------------------------------------
            inv_mask = main.tile([128, 128], F32)  # 1 - I
            nc.gpsimd.memset(inv_mask, 1.0)
            nc.gpsimd.affine_select(
                out=inv_mask, in_=inv_mask,
                compare_op=mybir.AluOpType.not_equal, fill=0.0,
                base=0, pattern=[[-1, 128]], channel_multiplier=1,
            )
            inv_mask_v = main.tile([128, 128], VT)  # 1 - I for pT tiles
            nc.gpsimd.memset(inv_mask_v, 1.0)
            nc.gpsimd.affine_select(
                out=inv_mask_v, in_=inv_mask_v,
                compare_op=mybir.AluOpType.not_equal, fill=0.0,
                base=0, pattern=[[-1, 128]], channel_multiplier=1,
            )
            ones_row = main.tile([1, 128], F32)  # lhsT for broadcasts / rhs ones
            nc.vector.memset(ones_row, 1.0)
            ident = main.tile([128, 128], F32)  # I, rhs of transpose matmuls
            nc.gpsimd.memset(ident, 0.0)
            nc.gpsimd.affine_select(
                out=ident, in_=ident,
                compare_op=mybir.AluOpType.not_equal, fill=1.0,
                base=0, pattern=[[-1, 128]], channel_multiplier=1,
            )

            # ---- persistent activations -----------------------------------
            # qhT/khT: [e, s] per-head-transposed projections; channel
            # 128*m + p lives at (partition p, free index m).
            qhT = main.tile([128, 2, s_len], MT)
            khT = main.tile([128, 2, s_len], MT)
            # vh: natural [s, e'] layout + per-head ones column.
            # Head h occupies cols 65h..65h+63; col 65h+64: ones (even h) /
            # col 65h: ones, 65h+1.. v (odd h).
            vh = main.tile([128, NT, AUG], VT)
            # attnT pair-packed: pair m holds head 2m (rows 0:64) and head
            # 2m+1 (rows 64:128), [e', q] layout.
            attnT = main.tile([128, 2, s_len], VT)
            wo_sb = main.tile([128, 2, d_model], VT)
            nc.sync.dma_start(
                out=wo_sb, in_=woT[:].rearrange("(m p) d -> p m d", p=128)
            )
            bq_sb = main.tile([128, 2], F32)
            nc.gpsimd.dma_start(out=bq_sb, in_=bqv[:].rearrange("(m p) -> p m", p=128))
            bk_sb = main.tile([128, 2], F32)
            nc.gpsimd.dma_start(out=bk_sb, in_=bkv[:].rearrange("(m p) -> p m", p=128))
            bv_sb = main.tile([128, AUG], F32)
            bvb_ap = bvb[:]
            nc.gpsimd.dma_start(
                out=bv_sb,
                in_=bass.AP(tensor=bvb_ap.tensor, offset=bvb_ap.offset,
                            ap=[[0, 128]] + list(bvb_ap.ap)),
            )

            # ---- projections ----------------------------------------------
            with tc.tile_pool(name="wpool", bufs=1) as wp, \
                 tc.tile_pool(name="xpool", bufs=3) as xp, \
                 tc.tile_pool(name="ppj", bufs=8, space="PSUM") as pp:
                wq_sb = wp.tile([128, KC, CPC], MT)
                nc.sync.dma_start(
                    out=wq_sb, in_=wqT[:].rearrange("(c p) e -> p c e", p=128))
                wk_sb = wp.tile([128, KC, CPC], MT)
                nc.sync.dma_start(
                    out=wk_sb, in_=wkT[:].rearrange("(c p) e -> p c e", p=128))
                wv_sb = wp.tile([128, KC, AUG], MT)
                nc.sync.dma_start(
                    out=wv_sb, in_=wvT[:].rearrange("(c p) e -> p c e", p=128))

                GW = s_len // XG  # columns per x-chunk group
                # q/k projections: out[e, s] = sum_d W[d, e] * xT[d, s]
                for xdram, w_sb, b_sb, outT in (
                    (qT, wq_sb, bq_sb, qhT), (kT, wk_sb, bk_sb, khT),
                ):
                    for grp in range(XG):
                        psl = []
                        for m in range(2):
                            for n in range(GW // 512):
                                pt = pp.tile([128, 512], F32, tag="pj",
                                             name=f"pj_{m}_{n}")
                                psl.append(pt)
                        for kk in range(KC):
                            xc = xp.tile([128, GW], MT, tag="xc", name="xc")
                            nc.sync.dma_start(
                                out=xc,
                                in_=xdram[kk * 128:(kk + 1) * 128,
                                          grp * GW:(grp + 1) * GW])
                            for m in range(2):
                                for n in range(GW // 512):
                                    nc.tensor.matmul(
                                        psl[m * (GW // 512) + n],
                                        lhsT=w_sb[:, kk, m * 128:(m + 1) * 128],
                                        rhs=xc[:, n * 512:(n + 1) * 512],
                                        start=(kk == 0), stop=(kk == KC - 1))
                        for m in range(2):
                            for n in range(GW // 512):
                                # DVE drain (keeps ACT free for the exps)
                                nc.vector.tensor_scalar_add(
                                    outT[:, m, grp * GW + n * 512:
                                         grp * GW + (n + 1) * 512],
                                    psl[m * (GW // 512) + n],
                                    b_sb[:, m:m + 1])
                # v projection: out[s, e'] = sum_d vT[d, s] * Wv[d, e'] + b
                for grp in range(XG):
                    pvl = [pp.tile([128, AUG], F32, tag="pj", name=f"pv_{st}")
                           for st in range(GW // 128)]
                    for kk in range(KC):
                        xc = xp.tile([128, GW], MT, tag="xc", name="xcv")
                        nc.sync.dma_start(
                            out=xc, in_=vT[kk * 128:(kk + 1) * 128,
                                           grp * GW:(grp + 1) * GW])
                        for st in range(GW // 128):
                            nc.tensor.matmul(
                                pvl[st],
                                lhsT=xc[:, st * 128:(st + 1) * 128],
                                rhs=wv_sb[:, kk, :],
                                start=(kk == 0), stop=(kk == KC - 1))
                    for st in range(GW // 128):
                        nc.vector.tensor_add(
                            vh[:, grp * (GW // 128) + st, :], pvl[st], bv_sb)

            # ---- attention -------------------------------------------------
            with tc.tile_pool(name="apool", bufs=1) as ap, \
                 tc.tile_pool(name="patt", bufs=1, space="PSUM") as pa:
                HW2 = s_len // 2          # q-half width for the B side
                CB = min(512, HW2)        # B-side psum chunk
                NCB = HW2 // CB
                for hp in range(2):
                    h0, h1 = 2 * hp, 2 * hp + 1
                    # ---- B side, head pair, two q-half passes ----------
                    # The two heads' K=64 scores matmuls sit at PE row
                    # groups 0 and 64 (auto tile_position) and issue
                    # back-to-back -> they execute concurrently.
                    z_ts, raw_ts = [], []
                    for hh in (h0, h1):
                        z_ts.append(ap.tile([65, s_len], F32, tag="zt",
                                            bufs=2, name=f"z_t{hh % 2}"))
                        raw_ts.append(ap.tile([64, s_len], F32, tag="ah",
                                              bufs=2, name=f"raw_t{hh % 2}"))
                    for half in range(2):
                        q0 = half * HW2
                        attn0 = pa.tile([65, HW2], F32, tag="attn", bufs=2,
                                        name="attn0")
                        attn1 = pa.tile([65, HW2], F32, tag="attn", bufs=2,
                                        name="attn1")
                        for lt in range(NT):
                            pT0 = ap.tile([128, HW2], VT, tag="pT", bufs=4,
                                          name="pT0")
                            pT1 = ap.tile([128, HW2], VT, tag="pT", bufs=4,
                                          name="pT1")
                            for n in range(NCB):
                                sc0 = pa.tile([128, CB], F32, tag="sc",
                                              bufs=4, name="sc0")
                                sc1 = pa.tile([128, CB], F32, tag="sc",
                                              bufs=4, name="sc1")
                                cc = q0 + n * CB
                                nc.tensor.matmul(
                                    sc0,
                                    lhsT=khT[0:64, hp, lt * 128:(lt + 1) * 128],
                                    rhs=qhT[0:64, hp, cc:cc + CB],
                                    start=True, stop=True)
                                nc.tensor.matmul(
                                    sc1,
                                    lhsT=khT[64:128, hp, lt * 128:(lt + 1) * 128],
                                    rhs=qhT[64:128, hp, cc:cc + CB],
                                    start=True, stop=True)
                                nc.scalar.activation(
                                    pT0[:, n * CB:(n + 1) * CB], sc0,
                                    AF.Exp, scale=0.125)
                                nc.scalar.activation(
                                    pT1[:, n * CB:(n + 1) * CB], sc1,
                                    AF.Exp, scale=0.125)
                            if q0 <= lt * 128 < q0 + HW2:
                                col = lt * 128 - q0
                                nc.vector.tensor_mul(
                                    pT0[:, col:col + 128],
                                    pT0[:, col:col + 128], inv_mask_v)
                                nc.vector.tensor_mul(
                                    pT1[:, col:col + 128],
                                    pT1[:, col:col + 128], inv_mask_v)
                            for n in range(HW2 // 512 if HW2 >= 512 else 1):
                                w2 = min(512, HW2)
                                nc.tensor.matmul(
                                    attn0[0:65, n * w2:(n + 1) * w2],
                                    lhsT=vh[:, lt, h0 * 65:(h0 + 1) * 65],
                                    rhs=pT0[:, n * w2:(n + 1) * w2],
                                    start=(lt == 0), stop=(lt == NT - 1))
                                nc.tensor.matmul(
                                    attn1[0:65, n * w2:(n + 1) * w2],
                                    lhsT=vh[:, lt, h1 * 65:(h1 + 1) * 65],
                                    rhs=pT1[:, n * w2:(n + 1) * w2],
                                    start=(lt == 0), stop=(lt == NT - 1))
                        # drain this q-half out of PSUM promptly
                        for i_h, attn_ps in ((0, attn0), (1, attn1)):
                            nc.vector.tensor_scalar_mul(
                                z_ts[i_h][64:65, q0:q0 + HW2],
                                attn_ps[64:65, :], 1.0 + EPS)
                            nc.vector.tensor_copy(
                                raw_ts[i_h][:, q0:q0 + HW2], attn_ps[0:64, :])

                    # ---- stats + attnT scale, per head -----------------
                    neg_lnds = []
                    for i_h, hh in enumerate((h0, h1)):
                        odd = hh % 2
                        z_t, raw_t = z_ts[i_h], raw_ts[i_h]
                        d_row = ap.tile([1, s_len], F32, tag="d0", bufs=1,
                                        name="d_row")
                        nc.sync.dma_start(out=d_row, in_=z_t[64:65, :])
                        # D -> partition layout; [128, NT] reciprocal (a
                        # [1, S] reciprocal runs 13us on one DVE lane)
                        dp_ps = pa.tile([128, CB], F32, tag="sc", bufs=4,
                                        name="dp_ps")
                        for t in range(NT):
                            nc.tensor.matmul(
                                dp_ps[:, t:t + 1],
                                lhsT=d_row[:, t * 128:(t + 1) * 128],
                                rhs=ones_row[:, 0:1], start=True, stop=True)
                        rdp = ap.tile([128, NT], F32, tag="rdp", bufs=2,
                                      name="rdp")
                        nc.vector.reciprocal(rdp, dp_ps[:, :NT])
                        neg_lnd = ap.tile([128, NT], F32, tag="nld", bufs=2,
                                          name="neg_lnd")
                        nc.scalar.activation(neg_lnd, rdp, AF.Ln)  # ln(1/D)
                        neg_lnds.append(neg_lnd)
                        # 1/D back to free layout for the 64-row broadcast
                        rd_row = ap.tile([1, s_len], F32, tag="rd", bufs=1,
                                         name="rd_row")
                        for hf in range(s_len // CB):
                            rf_ps = pa.tile([1, CB], F32, tag="sc", bufs=4,
                                            name="rf_ps")
                            for t8 in range(CB // 128):
                                t = hf * (CB // 128) + t8
                                nc.tensor.matmul(
                                    rf_ps[0:1, t8 * 128:(t8 + 1) * 128],
                                    lhsT=rdp[:, t:t + 1], rhs=ident,
                                    start=True, stop=True)
                            nc.vector.tensor_copy(
                                rd_row[:, hf * CB:(hf + 1) * CB], rf_ps)
                        bc_sb = ap.tile([64, s_len], F32, tag="bc", bufs=2,
                                        name="bc_sb")
                        for n in range(NQ):
                            bc_ps = pa.tile([64, 512], F32, tag="sc", bufs=4,
                                            name="bc_ps")
                            nc.tensor.matmul(
                                bc_ps, lhsT=ones_row[:, :64],
                                rhs=rd_row[:, n * 512:(n + 1) * 512],
                                start=True, stop=True)
                            nc.vector.tensor_copy(
                                bc_sb[:, n * 512:(n + 1) * 512], bc_ps)
                        if not odd:
                            nc.vector.tensor_mul(
                                attnT[0:64, hp, :], raw_t, bc_sb)
                        else:
                            ah_t = ap.tile([64, s_len], VT, tag="ah2", bufs=1,
                                           name="ah_t")
                            nc.vector.tensor_mul(ah_t, raw_t, bc_sb)
                            # DVE is lane-locked; DMA crosses partitions
                            nc.sync.dma_start(
                                out=attnT[64:128, hp, :], in_=ah_t)

                    # ---- A side, head pair, packed ---------------------
                    for qt in range(NT):
                        s_t0 = ap.tile([128, s_len], F32, tag="st", bufs=4,
                                       name="s_t0")
                        s_t1 = ap.tile([128, s_len], F32, tag="st", bufs=4,
                                       name="s_t1")
                        for n in range(NQ):
                            sa0 = pa.tile([128, 512], F32, tag="sc", bufs=4,
                                          name="sa0")
                            sa1 = pa.tile([128, 512], F32, tag="sc", bufs=4,
                                          name="sa1")
                            nc.tensor.matmul(
                                sa0,
                                lhsT=qhT[0:64, hp, qt * 128:(qt + 1) * 128],
                                rhs=khT[0:64, hp, n * 512:(n + 1) * 512],
                                start=True, stop=True)
                            nc.tensor.matmul(
                                sa1,
                                lhsT=qhT[64:128, hp, qt * 128:(qt + 1) * 128],
                                rhs=khT[64:128, hp, n * 512:(n + 1) * 512],
                                start=True, stop=True)
                            nc.scalar.activation(
                                s_t0[:, n * 512:(n + 1) * 512], sa0,
                                AF.Exp, scale=0.125,
                                bias=neg_lnds[0][:, qt:qt + 1])
                            nc.scalar.activation(
                                s_t1[:, n * 512:(n + 1) * 512], sa1,
                                AF.Exp, scale=0.125,
                                bias=neg_lnds[1][:, qt:qt + 1])
                        nc.vector.tensor_mul(
                            s_t0[:, qt * 128:(qt + 1) * 128],
                            s_t0[:, qt * 128:(qt + 1) * 128], inv_mask)
                        nc.vector.tensor_mul(
                            s_t1[:, qt * 128:(qt + 1) * 128],
                            s_t1[:, qt * 128:(qt + 1) * 128], inv_mask)
                        nc.sync.dma_start(
                            out=s_part[h0, qt * 128:(qt + 1) * 128, :],
                            in_=s_t0)
                        nc.sync.dma_start(
                            out=s_part[h1, qt * 128:(qt + 1) * 128, :],
                            in_=s_t1)

                # ---- output projection --------------------------------------
                for qt in range(NT):
                    o_t = ap.tile([128, d_model], F32, tag="st", bufs=3,
                                  name="o_t")
                    for n2 in range(d_model // 512):
                        op_ps = pa.tile([128, 512], F32, tag="sc", bufs=4,
                                        name="op_ps")
                        for m in range(2):
                            nc.tensor.matmul(
                                op_ps,
                                lhsT=attnT[:, m, qt * 128:(qt + 1) * 128],
                                rhs=wo_sb[:, m, n2 * 512:(n2 + 1) * 512],
                                start=(m == 0), stop=(m == 1))
                        nc.vector.tensor_copy(
                            o_t[:, n2 * 512:(n2 + 1) * 512], op_ps)
                    nc.sync.dma_start(
                        out=out_p[qt * 128:(qt + 1) * 128, :], in_=o_t)

    lowp.__exit__(None, None, None)
    nc.finalize()
    return nc


_CACHE: dict = {}


def _get_module():
    if "nc" not in _CACHE:
        _CACHE["nc"] = build_core_module()
    return _CACHE["nc"]


def _shard_inputs(q, k, v, Wq, bq, Wk, bk, Wv, bv, Wo, bo):
    """Build the 8 per-core input maps (host-side transposes/slices)."""
    f = np.float32
    import ml_dtypes
    mf = ml_dtypes.bfloat16 if KERNEL_DT == "bf16" else np.float32
    vf = ml_dtypes.bfloat16 if KERNEL_DT in ("bf16", "hybrid") else np.float32
    in_maps = []
    for c in range(8):
        b, g = divmod(c, 4)
        sel = slice(g * CPC, (g + 1) * CPC)
        wv_aug = np.zeros((D, AUG), f)
        bv_aug = np.zeros(AUG, f)
        for hh in range(HPC):
            rows = Wv[g * CPC + hh * HD:g * CPC + (hh + 1) * HD, :]  # [64, D]
            bl = bv[g * CPC + hh * HD:g * CPC + (hh + 1) * HD]
            wv_aug[:, 65 * hh:65 * hh + 64] = rows.T
            bv_aug[65 * hh:65 * hh + 64] = bl
            bv_aug[65 * hh + 64] = 1.0  # ones column -> Zm row of the PV psum
        in_maps.append({
            "qT": np.ascontiguousarray(q[b].T, mf),
            "kT": np.ascontiguousarray(k[b].T, mf),
            "vT": np.ascontiguousarray(v[b].T, mf),
            "wqT": np.ascontiguousarray(Wq[sel, :].T, mf),
            "wkT": np.ascontiguousarray(Wk[sel, :].T, mf),
            "wvT": wv_aug.astype(mf),
            "woT": np.ascontiguousarray(Wo[:, sel].T, vf),
            "bqv": np.ascontiguousarray(bq[sel], f),
            "bkv": np.ascontiguousarray(bk[sel], f),
            "bvb": bv_aug,
        })
    return in_maps


def kernel(q, k, v, Wq, bq, Wk, bk, Wv, bv, Wo, bo):
    q, k, v = (np.asarray(x, np.float32) for x in (q, k, v))
    Wq, bq, Wk, bk, Wv, bv, Wo, bo = (
        np.asarray(x, np.float32) for x in (Wq, bq, Wk, bk, Wv, bv, Wo, bo))

    from concourse.bass_utils import run_bass_kernel_spmd

    nc = _get_module()
    in_maps = _shard_inputs(q, k, v, Wq, bq, Wk, bk, Wv, bv, Wo, bo)
    res = run_bass_kernel_spmd(nc, in_maps, core_ids=list(range(8)))
    _CACHE["last_res"] = res  # exec_time_ns/profile when BASS_TRACE=1
    results = res.results

    s_full = np.empty((B, H, S, S), np.float32)
    out = np.empty((B, S, D), np.float32)
    for b in range(B):
        acc = None
        for g in range(4):
            r = results[b * 4 + g]
            s_full[b, g * HPC:(g + 1) * HPC] = r["s_part"]
            acc = r["out_p"].astype(np.float32) if acc is None else acc + r["out_p"]
        out[b] = acc + bo[None, :]
    return out, s_full


# head-group order within a core maps to global heads g*4 + hh
